# revision 34
# baseline (speedup 1.0000x reference)
"""MultiHeadAttention (B=2,N=2048,C=1024,H=16,Dk=64) on 8 TRN2 cores.

Head-tensor-parallel: core c owns heads {2c,2c+1} for both batches.
Device computes qkv^T = Wqkv_s^T @ x^T, causal softmax(q k^T/8) @ v, and the
partial out-projection (rows 128c:128c+128 of W_out); host sums the 8
partials (the "all-reduce"), transposes, and adds the fused bias.
b_k drops (softmax shift invariance); b_v folds into the output bias.
"""
import sys

sys.path.insert(0, "/opt/trn_rl_repo")
import numpy as np
import ml_dtypes
import concourse.bass as bass
import concourse.mybir as mybir
from concourse.bass_utils import run_bass_kernel_spmd
from concourse.tile import TileContext

F32 = mybir.dt.float32
F16 = mybir.dt.float16
BF16 = mybir.dt.bfloat16
AF = mybir.ActivationFunctionType
BF = ml_dtypes.bfloat16

T = 4096  # total tokens (2 batches x 2048)
TRACE = False
LAST_EXEC_NS = None
LAST_MEAN_NS = None

_MAX_WAITS = 1  # this neuronxcc build rejects instructions with more sem waits


def _split_excess_waits(nc, limit=_MAX_WAITS):
    """Move excess sem waits onto same-engine nops inserted just before the
    over-subscribed instruction (waits-before-inst on the same queue is
    semantically identical)."""
    ifaces = [nc.tensor, nc.scalar, nc.vector, nc.gpsimd, nc.sync]
    eng_map = {iface.engine: iface for iface in ifaces}
    f = nc.m.functions[0]
    for bb in list(f.blocks):
        il = bb.instructions
        i = 0
        while i < len(il):
            ins = il[i]
            si = ins.sync_info
            waits = list(si.on_wait) if si is not None else []
            if len(waits) > limit:
                keep = waits[-limit:]
                rest = waits[:-limit]
                ins.sync_info = mybir.SyncInfo(
                    on_wait=keep, on_update=list(si.on_update)
                )
                nops = []
                for k in range(0, len(rest), limit):
                    nop = eng_map[ins.engine].nop(nofuse=True)
                    nop.ins.sync_info = mybir.SyncInfo(
                        on_wait=rest[k : k + limit], on_update=[]
                    )
                    nops.append(nop.ins)
                for ni in nops:
                    for bb2 in list(f.blocks):
                        try:
                            bb2.instructions.remove(ni)
                            break
                        except ValueError:
                            pass
                for off, ni in enumerate(nops):
                    il.insert(i + off, ni)
                i += len(nops)
            i += 1


def _build():
    nc = bass.Bass("TRN2", target_bir_lowering=False, debug=False, num_devices=8)
    xt_d = nc.declare_dram_parameter("xt", (1024, T), BF16, isOutput=False)
    wqkv_d = nc.declare_dram_parameter("wqkv", (1024, 384), BF16, isOutput=False)
    bq_d = nc.declare_dram_parameter("bq", (128, 1), F32, isOutput=False)
    wout_d = nc.declare_dram_parameter("wout", (128, 1024), BF16, isOutput=False)
    tri_d = nc.declare_dram_parameter("tri", (128, 128), BF16, isOutput=False)
    sel_d = nc.declare_dram_parameter("sel", (16, 1024), BF16, isOutput=False)
    ident_d = nc.declare_dram_parameter("ident", (128, 128), BF16, isOutput=False)
    outp_d = nc.declare_dram_parameter("outp", (1024, T), F16, isOutput=True)

    with TileContext(nc) as tc:
        with tc.tile_pool(name="sb", bufs=1) as sb:
            # ---- constant / persistent tiles + input DMAs ----
            # two queues, inputs fully landed before compute: overlapping the
            # xt stream with phase A measured ~20% slower on every engine
            wq_t = [
                sb.tile((128, 384), BF16, tag=f"wq{kc}", name=f"wq{kc}")
                for kc in range(8)
            ]
            xt_t = [
                sb.tile((128, T), BF16, tag=f"xt{kc}", name=f"xt{kc}")
                for kc in range(8)
            ]
            bq_t = sb.tile((128, 1), F32, tag="bq")
            wout_t = sb.tile((128, 1024), BF16, tag="wout")
            tri_t = sb.tile((128, 128), BF16, tag="tri")
            id_t = sb.tile((128, 128), BF16, tag="ident")
            S_all = sb.tile((16, 1024), BF16, tag="sel")

            for kc in range(8):
                nc.sync.dma_start(wq_t[kc][:], wqkv_d[128 * kc : 128 * kc + 128, :])
            nc.gpsimd.dma_start(bq_t[:], bq_d[:, :])
            nc.gpsimd.dma_start(wout_t[:], wout_d[:, :])
            nc.gpsimd.dma_start(tri_t[:], tri_d[:, :])
            nc.gpsimd.dma_start(id_t[:], ident_d[:, :])
            for kc in range(8):
                eng = nc.sync if kc % 2 == 0 else nc.gpsimd
                eng.dma_start(xt_t[kc][:], xt_d[128 * kc : 128 * kc + 128, :])
            nc.gpsimd.dma_start(S_all[:], sel_d[:, :])

            q_T = sb.tile((128, T), BF16, tag="q_T")
            k_T = sb.tile((128, T), BF16, tag="k_T")
            v_T = sb.tile((128, T), BF16, tag="v_T")
            vext = [
                sb.tile((128, 2080), BF16, tag=f"vext{b}", name=f"vext{b}")
                for b in range(2)
            ]
            nc.vector.memset(vext[0][:], 1.0)
            nc.vector.memset(vext[1][:], 1.0)

            # es buffers for diagonal key-blocks: fully-masked columns are
            # zeroed once and never rewritten (exp writes only unmasked cols)
            es_diag = [
                sb.tile((128, 1024), BF16, tag=f"esd{r}", name=f"esd{r}")
                for r in range(4)
            ]
            for r in range(1, 4):
                nc.gpsimd.memset(es_diag[r][:, 0 : 128 * r], 0.0)
                nc.gpsimd.memset(es_diag[r][:, 512 : 512 + 128 * r], 0.0)

            # unnormalized attention outputs (row 64 = softmax denominator)
            av_sb = [
                sb.tile((65, 1024), F32, tag=f"avsb{t}", name=f"avsb{t}")
                for t in range(8)
            ]
            den16 = sb.tile((16, 512), F32, tag="den16")
            rec16 = sb.tile((16, 512), F32, tag="rec16")

            # ---- phase A: qkv^T = wqkv_s^T @ x^T ----
            with tc.tile_pool(name="psA", bufs=1, space="PSUM") as psA:
                dst = [q_T, k_T, v_T]
                for m in range(3):
                    chs = [
                        psA.tile((128, 512), F32, tag=f"ch{n}", name=f"ch{n}")
                        for n in range(8)
                    ]
                    for kc in range(8):
                        for n in range(8):
                            nc.tensor.matmul(
                                chs[n][:],
                                wq_t[kc][:, 128 * m : 128 * m + 128],
                                xt_t[kc][:, 512 * n : 512 * n + 512],
                                start=(kc == 0),
                                stop=(kc == 7),
                            )
                    for n in range(8):
                        o = dst[m][:, 512 * n : 512 * n + 512]
                        if m == 0:
                            nc.vector.tensor_scalar_add(o, chs[n][:], bq_t[:, 0:1])
                        else:
                            nc.scalar.activation(o, chs[n][:], AF.Copy)

            # ---- phase V: transpose v_T into [token, dim] blocks with a
            # trailing ones column per 65-wide block (softmax denominator) ----
            with tc.tile_pool(name="psV", bufs=1, space="PSUM") as psV:
                for t in range(32):
                    b, jj = divmod(t, 16)
                    trp = psV.tile((128, 128), BF16, tag="tr", bufs=2)
                    nc.tensor.transpose(trp[:], v_T[:, 128 * t : 128 * t + 128], id_t[:])
                    # single DVE copy lands both halves: out chunks at 65*jj
                    # and 65*(16+jj) (stride 1040), keeping ACT free for exps
                    c0 = 65 * jj
                    oslc = vext[b][:, c0 : c0 + 64]
                    islc = trp[:]
                    o_ap = bass.AP(
                        oslc.tensor,
                        oslc.offset,
                        [[oslc.ap[0][0], oslc.ap[0][1]], [1040, 2], [1, 64]],
                    )
                    i_ap = bass.AP(
                        islc.tensor,
                        islc.offset,
                        [[islc.ap[0][0], islc.ap[0][1]], [64, 2], [1, 64]],
                    )
                    nc.vector.tensor_copy(o_ap, i_ap)

            # ---- phase B: causal attention (unnormalized) ----
            with tc.tile_pool(name="psB", bufs=1, space="PSUM") as psB:
                for t in range(8):
                    b, i = divmod(t, 4)
                    av = [
                        psB.tile(
                            (65, 512), F32, tag=f"av{hl}", name=f"av{hl}", bufs=2
                        )
                        for hl in range(2)
                    ]
                    nj = 4 * i + 4
                    qs = 2048 * b + 512 * i
                    for jj in range(nj):
                        sps = psB.tile((128, 1024), F32, tag="sps", bufs=2)
                        ks = 2048 * b + 128 * jj
                        for hl in range(2):
                            nc.tensor.matmul(
                                sps[:, 512 * hl : 512 * hl + 512],
                                k_T[64 * hl : 64 * hl + 64, ks : ks + 128],
                                q_T[64 * hl : 64 * hl + 64, qs : qs + 512],
                                start=True,
                                stop=True,
                                skip_group_check=True,
                            )
                        r = jj - 4 * i
                        if r < 0:
                            es = sb.tile((128, 1024), BF16, tag="es", bufs=3)
                            nc.scalar.activation(es[:], sps[:], AF.Exp, scale=0.125)
                        else:
                            es = es_diag[r]
                            if r == 0:
                                nc.scalar.activation(
                                    es[:], sps[:], AF.Exp, scale=0.125
                                )
                            else:
                                # one 2-chunk ACT call covers both hl halves
                                w = 512 - 128 * r
                                oslc = es[:, 128 * r : 128 * r + w]
                                islc = sps[:, 128 * r : 128 * r + w]
                                o_ap = bass.AP(
                                    oslc.tensor,
                                    oslc.offset,
                                    [
                                        [oslc.ap[0][0], oslc.ap[0][1]],
                                        [512, 2],
                                        [1, w],
                                    ],
                                )
                                i_ap = bass.AP(
                                    islc.tensor,
                                    islc.offset,
                                    [
                                        [islc.ap[0][0], islc.ap[0][1]],
                                        [512, 2],
                                        [1, w],
                                    ],
                                )
                                nc.scalar.activation(
                                    o_ap, i_ap, AF.Exp, scale=0.125
                                )
                            for hl in range(2):
                                c0 = 512 * hl + 128 * r
                                nc.vector.tensor_mul(
                                    es[:, c0 : c0 + 128],
                                    es[:, c0 : c0 + 128],
                                    tri_t[:],
                                )
                        for hl in range(2):
                            c = 65 * (16 * hl + jj)
                            nc.tensor.matmul(
                                av[hl][:],
                                vext[b][:, c : c + 65],
                                es[:, 512 * hl : 512 * hl + 512],
                                start=(jj == 0),
                                stop=(jj == nj - 1),
                                skip_group_check=True,
                            )
                    # evacuate + collect denominators (row 64) -- SBUF->SBUF DMA
                    # because compute engines need quadrant-aligned partitions
                    for hl in range(2):
                        nc.vector.tensor_copy(
                            av_sb[t][:, 512 * hl : 512 * hl + 512], av[hl][:]
                        )
                        eng = nc.sync if hl == 0 else nc.gpsimd
                        eng.dma_start(
                            den16[2 * t + hl : 2 * t + hl + 1, :],
                            av_sb[t][64:65, 512 * hl : 512 * hl + 512],
                        )
                nc.vector.reciprocal(rec16[:], den16[:])
                # Dekker hi/lo split so the broadcast matmul can run in bf16
                # (two accumulating passes) at full fp32 broadcast accuracy
                rec_hi = sb.tile((16, 512), BF16, tag="rec_hi")
                nc.vector.tensor_copy(rec_hi[:], rec16[:])
                rec_hif = sb.tile((16, 512), F32, tag="rec_hif")
                nc.scalar.activation(rec_hif[:], rec_hi[:], AF.Copy)
                rec_lo = sb.tile((16, 512), BF16, tag="rec_lo")
                nc.vector.tensor_sub(rec_lo[:], rec16[:], rec_hif[:])

            # ---- phase C: normalize + partial out-projection ----
            with tc.tile_pool(name="psC", bufs=1, space="PSUM") as psC:
                for t in range(8):
                    qs = 512 * t
                    bcp = psC.tile((128, 512), F32, tag="bcp", bufs=2)
                    nc.tensor.matmul(
                        bcp[:],
                        S_all[:, 128 * t : 128 * t + 128],
                        rec_hi[:],
                        start=True,
                        stop=False,
                        skip_group_check=True,
                    )
                    nc.tensor.matmul(
                        bcp[:],
                        S_all[:, 128 * t : 128 * t + 128],
                        rec_lo[:],
                        start=False,
                        stop=True,
                        skip_group_check=True,
                    )
                    attnT = sb.tile((128, 512), BF16, tag="attnT", bufs=2)
                    for hl in range(2):
                        nc.vector.tensor_mul(
                            attnT[64 * hl : 64 * hl + 64, :],
                            av_sb[t][0:64, 512 * hl : 512 * hl + 512],
                            bcp[64 * hl : 64 * hl + 64, :],
                        )
                    for mo in range(8):
                        op = psC.tile((128, 512), F32, tag="op", bufs=2)
                        nc.tensor.matmul(
                            op[:],
                            wout_t[:, 128 * mo : 128 * mo + 128],
                            attnT[:],
                            start=True,
                            stop=True,
                            skip_group_check=True,
                        )
                        osb = sb.tile((128, 512), F16, tag="osb", bufs=4)
                        # 3/5 DVE/ACT split: DVE also carries the attnT muls
                        if mo in (0, 3, 6):
                            nc.vector.tensor_copy(osb[:], op[:])
                        else:
                            nc.scalar.activation(osb[:], op[:], AF.Copy)
                        eng = nc.sync if mo % 2 == 0 else nc.gpsimd
                        eng.dma_start(
                            outp_d[128 * mo : 128 * mo + 128, qs : qs + 512],
                            osb[:],
                        )
    _split_excess_waits(nc)
    return nc


def kernel(**inputs):
    global LAST_EXEC_NS, LAST_MEAN_NS
    x = np.asarray(inputs["x"], np.float32)
    Wqkv = np.asarray(inputs["W_qkv"], np.float32)
    bqkv = np.asarray(inputs["b_qkv"], np.float32)
    Wout = np.asarray(inputs["W_out"], np.float32)
    bout = np.asarray(inputs["b_out"], np.float32)

    xt = np.ascontiguousarray(x.reshape(T, 1024).T).astype(BF)
    kk = np.arange(128)[:, None]
    qq = np.arange(128)[None, :]
    tri = (qq >= kk).astype(BF)
    ident = np.eye(128).astype(BF)
    sel = np.zeros((16, 1024), BF)
    for t in range(8):
        sel[2 * t, 128 * t : 128 * t + 64] = 1.0
        sel[2 * t + 1, 128 * t + 64 : 128 * t + 128] = 1.0

    in_maps = []
    for c in range(8):
        s = 128 * c
        wq = np.ascontiguousarray(
            np.concatenate(
                [
                    Wqkv[:, s : s + 128],
                    Wqkv[:, 1024 + s : 1024 + s + 128],
                    Wqkv[:, 2048 + s : 2048 + s + 128],
                ],
                axis=1,
            )
        ).astype(BF)
        in_maps.append(
            {
                "xt": xt,
                "wqkv": wq,
                "bq": np.ascontiguousarray(
                    bqkv[s : s + 128].reshape(128, 1)
                ).astype(np.float32),
                "wout": np.ascontiguousarray(Wout[s : s + 128, :]).astype(BF),
                "tri": tri,
                "sel": sel,
                "ident": ident,
            }
        )

    nc = _build()
    res = run_bass_kernel_spmd(nc, in_maps, list(range(8)), trace=TRACE)
    LAST_EXEC_NS = res.exec_time_ns
    LAST_MEAN_NS = res.mean_exec_time_ns

    total = np.zeros((1024, T), np.float32)
    for c in range(8):
        total += np.asarray(res.results[c]["outp"]).astype(np.float32)
    beff = (
        bout.astype(np.float64) + bqkv[2048:].astype(np.float64) @ Wout.astype(np.float64)
    ).astype(np.float32)
    out = total.T.reshape(2, 2048, 1024) + beff
    return out.astype(np.float32)
